# revision 42
# baseline (speedup 1.0000x reference)
"""Trainium2 Bass kernel: RMSNorm + RoPE + causal attention + output projection.

Tensor-parallel over heads: 16 heads / 8 cores = 2 heads per core.
Each core computes a full [S, D] partial output; the all-reduce is done
host-side in the gather.

v5 design (549us -> 444us vs v3; all-bf16 matmuls except fp8 gram):
  - PV-aug: the PV matmul uses pt blocks as the STATIONARY operand and
    [V | 1]-augmented vn as the moving operand, so the softmax denominator
    Z falls out as the 129th output column -- the separate ones-matmul for
    Z is eliminated (~-60us PE). LDWEIGHTS pipelines under the 129-col
    streams at ~57ns/matmul. PSUM zero-regions are BANK-granular (2KB):
    exactly one start/stop per bank across the interleaved qb groups.
  - The [q, hd] attention blocks are normalized with a per-partition
    1/Z (reciprocal_approx_fast on [128,1]) and transposed back to
    [hd, q] by PE identity matmuls injected into the pair streams.
  - RMSNorm stats: fp8e4 DoubleRow gram on a host-quantized x^T copy
    (istd is insensitive to fp8 error), diag extracted with fused DVE ops,
    bit-hack rsqrt.
  - V projection emitted in natural [s, hd] layout (ht stationary, wv
    moving): kills the PE transposes and the vstage scalar copy; istd is a
    per-partition scalar multiply on the drain.
  - wo^T comes transposed from the host; no on-device wot build.
  - QKV projection matmuls of chunk c+1 are interleaved into chunk c's
    attention pair stream (V into head0, Q into head1, K into the output
    projection): the attention phase is scalar-rate-bound (exp ~1.1us/pair
    vs PE ~0.85us/pair), the injected matmuls absorb the difference.
  - All DRAM tensors use chunk-major host layouts so every DMA is one
    contiguous descriptor (the (t p)->p t rearranges cost ~0.6us/descr).
  - Out-proj drains split scalar/DVE; output in bf16 (host sums partials
    in fp32), DMAs alternate sync / gpsimd queues.
  - PSUM: bigp 2x[128,1024] (sp/gram/bcast/atT/op), ozA 1x[128,1024]
    (oz0/oz1/pp_K), ppp 1x[128,1024] (pp_V/pp_Q) = 8 banks.
"""
import os
import sys
import types

import numpy as np
import ml_dtypes

SEQ, D, NH, HD = 4096, 2048, 16, 128
NCORES = 8
HPC = NH // NCORES          # heads per core
M = HPC * HD                # per-core fused head dim (256)
EPS = 1e-6
ROPE_BASE = 10000.0
SM_SCALE = 1.0 / np.sqrt(HD)
CHUNK = 512                 # q-chunk
NCHUNK = SEQ // CHUNK       # 8
DT = D // 128               # 16 d-tiles
X8_SCALE = 32.0             # host fp8 scale for xsT8 (power of 2)
NWARM = 20


def _inject_ntff_hook():
    """Register the axon NTFF profiling hook (missing antenv.axon_hooks)."""
    if "antenv.axon_hooks" in sys.modules:
        return
    try:
        import antenv
        from trn_agent_boot.trn_boot import _ntff_profile_via_ctypes
    except ImportError:
        return
    holder = [None]
    mod = types.ModuleType("antenv.axon_hooks")
    mod.set_axon_ntff_profile_hook = lambda h: holder.__setitem__(0, h)
    mod.get_axon_ntff_profile_hook = lambda: holder[0]
    sys.modules["antenv.axon_hooks"] = mod
    antenv.axon_hooks = mod
    try:
        mod.set_axon_ntff_profile_hook(
            _ntff_profile_via_ctypes("/opt/axon/libaxon_pjrt.so"))
    except Exception:
        pass


def _build_nc():
    import concourse.bass as bass  # noqa: F401
    import concourse.mybir as mybir
    import concourse.tile as tile
    from concourse import bacc

    FP32 = mybir.dt.float32
    FP32R = mybir.dt.float32r
    BF16 = mybir.dt.bfloat16
    F8 = mybir.dt.float8e4
    U32 = mybir.dt.uint32
    AF = mybir.ActivationFunctionType
    ALU = mybir.AluOpType
    DR = mybir.MatmulPerfMode.DoubleRow

    nc = bacc.Bacc(None, target_bir_lowering=False)

    # chunk-major host layouts: one contiguous DMA descriptor per transfer
    xsT = nc.declare_dram_parameter("xsT", [NCHUNK * 128, DT * CHUNK], BF16,
                                    isOutput=False)
    xsT8 = nc.declare_dram_parameter("xsT8", [NCHUNK * 128, DT * CHUNK], F8,
                                     isOutput=False)
    wq = nc.declare_dram_parameter("wq", [128, DT * M], BF16, isOutput=False)
    wk = nc.declare_dram_parameter("wk", [128, DT * M], BF16, isOutput=False)
    wv = nc.declare_dram_parameter("wv", [128, DT * M], BF16, isOutput=False)
    wod = nc.declare_dram_parameter("wod", [M, D], BF16, isOutput=False)
    cosd = nc.declare_dram_parameter("cosd", [128, SEQ], BF16, isOutput=False)
    sind = nc.declare_dram_parameter("sind", [128, SEQ], BF16, isOutput=False)
    tri = nc.declare_dram_parameter("tri", [128, 128], BF16, isOutput=False)
    ident = nc.declare_dram_parameter("ident", [128, 128], BF16, isOutput=False)
    identr = nc.declare_dram_parameter("identr", [128, 128], FP32R,
                                       isOutput=False)
    sel4 = nc.declare_dram_parameter("sel4", [4, 512], FP32R, isOutput=False)
    out = nc.declare_dram_parameter("out", [SEQ, D], BF16, isOutput=True)

    with tile.TileContext(nc) as tc:
        with tc.tile_pool(name="consts", bufs=1) as consts, \
             tc.tile_pool(name="state", bufs=1) as state, \
             tc.tile_pool(name="ht", bufs=2) as htp, \
             tc.tile_pool(name="ht8", bufs=2) as ht8p, \
             tc.tile_pool(name="nrm", bufs=3) as nrm, \
             tc.tile_pool(name="sct", bufs=3) as sct, \
             tc.tile_pool(name="qtc", bufs=2) as qtcp, \
             tc.tile_pool(name="rp", bufs=2) as rpp, \
             tc.tile_pool(name="pt", bufs=5) as ptp, \
             tc.tile_pool(name="rz", bufs=2) as rzp, \
             tc.tile_pool(name="att", bufs=3) as attp, \
             tc.tile_pool(name="ost", bufs=3) as ostp, \
             tc.tile_pool(name="big", bufs=2, space="PSUM") as big, \
             tc.tile_pool(name="ozA", bufs=1, space="PSUM") as ozA, \
             tc.tile_pool(name="ppp", bufs=1, space="PSUM") as ppp:

            # ---- warmup: keep PE busy during the initial DMA window ----
            junk = consts.tile([128, 512], BF16)
            nc.vector.memset(junk[:], 0.125)
            for _ in range(NWARM):
                wm = big.tile([128, 512], FP32, name="warm", tag="big")
                nc.tensor.matmul(wm[:], junk[:, 0:128], junk[:],
                                 start=True, stop=True)

            # ---- setup DMAs: activations/weights on sync; consts and
            # tables on the scalar queue (idle before the first exp) ----
            def emit_ht8_dma(c, pieces=1):
                t8 = ht8p.tile([128, DT, CHUNK], F8, name="ht8")
                q = DT // pieces
                for pz in range(pieces):
                    nc.sync.dma_start(
                        out=t8[:, pz * q:(pz + 1) * q, :],
                        in_=xsT8[c * 128:(c + 1) * 128,
                                 pz * q * CHUNK:(pz + 1) * q * CHUNK
                                 ].rearrange("p (t s) -> p t s", t=q))
                return t8

            def emit_ht_dma(c, pieces=1):
                ht = htp.tile([128, DT, CHUNK], BF16, name="ht")
                q = DT // pieces
                for pz in range(pieces):
                    nc.sync.dma_start(
                        out=ht[:, pz * q:(pz + 1) * q, :],
                        in_=xsT[c * 128:(c + 1) * 128,
                                pz * q * CHUNK:(pz + 1) * q * CHUNK
                                ].rearrange("p (t s) -> p t s", t=q))
                return ht

            ht8_cur = emit_ht8_dma(0, pieces=4)
            wq_sb = consts.tile([128, DT * M], BF16)
            nc.sync.dma_start(out=wq_sb[:], in_=wq[:])
            ht_cur = emit_ht_dma(0, pieces=4)
            wk_sb = consts.tile([128, DT * M], BF16)
            nc.sync.dma_start(out=wk_sb[:], in_=wk[:])
            ht8_nxt = emit_ht8_dma(1)
            wv_sb = consts.tile([128, DT * M], BF16)
            nc.sync.dma_start(out=wv_sb[:], in_=wv[:])
            ht_nxt = emit_ht_dma(1)

            ident_sb = consts.tile([128, 128], BF16)
            nc.scalar.dma_start(out=ident_sb[:], in_=ident[:])
            idr_sb = consts.tile([128, 128], FP32R)
            nc.scalar.dma_start(out=idr_sb[:], in_=identr[:])
            sel_sb = consts.tile([4, 512], FP32R)
            nc.scalar.dma_start(out=sel_sb[:], in_=sel4[:])
            cos_sb = consts.tile([128, SEQ], BF16)
            nc.scalar.dma_start(out=cos_sb[:], in_=cosd[:])
            sin_sb = consts.tile([128, SEQ], BF16)
            nc.scalar.dma_start(out=sin_sb[:], in_=sind[:])
            tri_sb = consts.tile([128, 128], BF16)
            nc.scalar.dma_start(out=tri_sb[:], in_=tri[:])
            wot = []
            for h in range(HPC):
                w_ = consts.tile([128, D], BF16, name=f"wot{h}")
                nc.scalar.dma_start(out=w_[:],
                                    in_=wod[h * 128:(h + 1) * 128, :])
                wot.append(w_)
            magic_sb = consts.tile([128, 4], U32)
            nc.vector.memset(magic_sb[:], 0x5F3759DF)

            # persistent per-head state; vn is [V | 1] augmented so the PV
            # matmul's 129th moving column produces the softmax denominator
            kt = [state.tile([128, SEQ], BF16, name=f"kt{h}") for h in range(HPC)]
            vn = [state.tile([128, SEQ // 128, HD + 4], BF16, name=f"vn{h}")
                  for h in range(HPC)]
            for h in range(HPC):
                nc.vector.memset(vn[h][:, :, HD:HD + 1], 1.0)

            # ---- norm stats: fp8 DoubleRow gram -> istd4 (bit-hack rsqrt) --
            def emit_stats_gram(c, ht8):
                gram = big.tile([128, 512], FP32, name="gram", tag="big")
                for st in range(4):
                    for kp in range(DT // 2):
                        blk = ht8[:, 2 * kp:2 * kp + 2, st * 128:(st + 1) * 128]
                        nc.tensor.matmul(gram[:, st * 128:(st + 1) * 128],
                                         blk, blk, start=(kp == 0),
                                         stop=(kp == DT // 2 - 1), perf_mode=DR)
                scr = nrm.tile([128, 128], BF16, name="scr")
                ssq4 = nrm.tile([128, 4], FP32, name="ssq4")
                for st in range(4):
                    nc.vector.scalar_tensor_tensor(
                        out=scr[:], in0=gram[:, st * 128:(st + 1) * 128],
                        scalar=1.0, in1=ident_sb[:],
                        op0=ALU.mult, op1=ALU.mult,
                        accum_out=ssq4[:, st:st + 1])
                # istd = rsqrt(ssq/(D*X8^2) + eps): bit-hack + 2 Newton iters
                ms = nrm.tile([128, 4], FP32, name="ms")
                nc.vector.tensor_scalar(out=ms[:], in0=ssq4[:],
                                        scalar1=1.0 / (D * X8_SCALE * X8_SCALE),
                                        scalar2=EPS,
                                        op0=ALU.mult, op1=ALU.add)
                ih = nrm.tile([128, 4], U32, name="ih")
                nc.vector.tensor_scalar(out=ih[:], in0=ms[:].bitcast(U32),
                                        scalar1=1, scalar2=None,
                                        op0=ALU.logical_shift_right)
                y = nrm.tile([128, 4], FP32, name="y")
                nc.vector.scalar_tensor_tensor(
                    out=y[:].bitcast(U32), in0=magic_sb[:], scalar=0,
                    in1=ih[:], op0=ALU.bypass, op1=ALU.subtract)
                t = nrm.tile([128, 4], FP32, name="t")
                istd4 = nrm.tile([128, 4], FP32R, name="istd4")
                for it in range(2):
                    nc.vector.tensor_tensor(out=t[:], in0=y[:], in1=y[:],
                                            op=ALU.mult)
                    nc.vector.tensor_tensor(out=t[:], in0=t[:], in1=ms[:],
                                            op=ALU.mult)
                    nc.vector.tensor_scalar(out=t[:], in0=t[:], scalar1=-0.5,
                                            scalar2=1.5, op0=ALU.mult,
                                            op1=ALU.add)
                    dst = y[:] if it == 0 else istd4[:]
                    with nc.allow_low_precision(reason="istd bcast chain"):
                        nc.vector.tensor_tensor(out=dst, in0=y[:], in1=t[:],
                                                op=ALU.mult)
                return istd4

            # ---- istd broadcast [s]->[128, s-chunk] + scaled rope tables --
            def emit_stats_bcast(c, istd4):
                csl = slice(c * CHUNK, (c + 1) * CHUNK)
                itT = big.tile([4, 128], FP32R, name="itT", tag="big")
                nc.tensor.transpose(itT[:], istd4[:], idr_sb[:])
                itT_sb = nrm.tile([4, 128], FP32R, name="itT_sb")
                nc.vector.tensor_copy(itT_sb[:], itT[:])
                bc = big.tile([128, 512], FP32, name="bc", tag="big")
                for st in range(4):
                    nc.tensor.matmul(bc[:, st * 128:(st + 1) * 128],
                                     sel_sb[:, st * 128:(st + 1) * 128],
                                     itT_sb[:], start=True, stop=True)
                cos_sc = sct.tile([128, CHUNK], BF16, name="cos_sc")
                nc.vector.tensor_tensor(out=cos_sc[:], in0=bc[:],
                                        in1=cos_sb[:, csl], op=ALU.mult)
                sin_sc = sct.tile([128, CHUNK], BF16, name="sin_sc")
                nc.vector.tensor_tensor(out=sin_sc[:], in0=bc[:],
                                        in1=sin_sb[:, csl], op=ALU.mult)
                return cos_sc, sin_sc, istd4

            # ---- projection matmul fillers (emitted into pair streams) ----
            def qk_proj_fillers(pp, w_sb, ht):
                fillers = []
                for h in range(HPC):
                    for dt in range(DT):
                        def f(h=h, dt=dt):
                            nc.tensor.matmul(
                                pp[:, h * 512:(h + 1) * 512],
                                w_sb[:, dt * M + h * HD:dt * M + h * HD + 128],
                                ht[:, dt, :],
                                start=(dt == 0), stop=(dt == DT - 1))
                        fillers.append(f)
                return fillers

            def v_proj_fillers(pp, ht):
                # natural layout: out [s(128) x 256] per s-block
                fillers = []
                for sb in range(4):
                    for dt in range(DT):
                        def f(sb=sb, dt=dt):
                            nc.tensor.matmul(
                                pp[:, sb * 256:(sb + 1) * 256],
                                ht[:, dt, sb * 128:(sb + 1) * 128],
                                wv_sb[:, dt * M:(dt + 1) * M],
                                start=(dt == 0), stop=(dt == DT - 1))
                        fillers.append(f)
                return fillers

            # ---- rope drain for one projection (q or k) ----
            def emit_rope(c, pp, stats, kind):
                cos_sc, sin_sc, _ = stats
                qt_c = []
                for h in range(HPC):
                    hsl = slice(h * 512, (h + 1) * 512)
                    if kind == "q":
                        dst_t = qtcp.tile([128, CHUNK], BF16, name=f"qt{h}")
                        qt_c.append(dst_t)
                        dst = dst_t[:]
                    else:
                        dst = kt[h][:, c * CHUNK:(c + 1) * CHUNK]
                    pc = rpp.tile([128, CHUNK], FP32, name="pc")
                    nc.vector.tensor_tensor(
                        out=pc[:], in0=pp[:, hsl], in1=cos_sc[:], op=ALU.mult)
                    psw = rpp.tile([128, CHUNK], FP32, name="psw")
                    nc.vector.tensor_tensor(
                        out=psw[0:64, :], in0=pp[64:128, hsl],
                        in1=sin_sc[0:64, :], op=ALU.mult)
                    nc.vector.tensor_tensor(
                        out=psw[64:128, :], in0=pp[0:64, hsl],
                        in1=sin_sc[64:128, :], op=ALU.mult)
                    nc.vector.tensor_tensor(
                        out=dst, in0=pc[:], in1=psw[:], op=ALU.add)
                return qt_c

            # ---- V drain: per-partition istd multiply into vn ----
            def emit_v_drain(c, pp, istd4):
                for sb in range(4):
                    blk = 4 * c + sb
                    for h in range(HPC):
                        nc.vector.tensor_scalar_mul(
                            vn[h][:, blk, 0:HD],
                            pp[:, sb * 256 + h * 128:sb * 256 + (h + 1) * 128],
                            istd4[:, sb:sb + 1].bitcast(FP32))

            # ---- attention core for one head; fillers interleaved ----
            def emit_c_core(c, h, qt_c, oz, fillers):
                npair = 2 * c + 2
                jmax = 4 * c + 3
                LAGP = 3
                pend = {}
                nfill = len(fillers)
                fi = 0
                for p in range(npair + LAGP):
                    if p < npair:
                        sp = big.tile([128, 1024], FP32, name="sp", tag="big")
                        for i in range(2):
                            j = 2 * p + i
                            off = max(j - 4 * c, 0) * 128
                            nc.tensor.matmul(
                                sp[:, i * 512 + off:(i + 1) * 512],
                                kt[h][:, j * 128:(j + 1) * 128],
                                qt_c[h][:, off:], start=True, stop=True)
                        pt = ptp.tile([128, 1024], BF16, name="pt")
                        nc.scalar.activation(pt[:], sp[:], AF.Exp,
                                             scale=float(SM_SCALE))
                        for i in range(2):
                            r = 2 * p + i - 4 * c
                            if r >= 0:
                                off = i * 512 + r * 128
                                nc.vector.tensor_tensor(
                                    out=pt[:, off:off + 128],
                                    in0=pt[:, off:off + 128],
                                    in1=tri_sb[:], op=ALU.mult)
                        pend[p] = pt
                    if p >= LAGP:
                        pt = pend.pop(p - LAGP)
                        for i in range(2):
                            j = 2 * (p - LAGP) + i
                            offb = max(j - 4 * c, 0)
                            for qb in range(offb, 4):
                                # PSUM zero-regions are bank-wide (2KB):
                                # exactly one start per bank (first matmul
                                # touching it) and one stop (last)
                                nc.tensor.matmul(
                                    oz[:, qb * 256:qb * 256 + HD + 1],
                                    pt[:, i * 512 + qb * 128:
                                       i * 512 + (qb + 1) * 128],
                                    vn[h][:, j, 0:HD + 1],
                                    start=(j == 0 and qb % 2 == 0),
                                    stop=(qb % 2 == 1 and j == 4 * c + qb),
                                    skip_group_check=True)
                    # drain fillers uniformly across iterations
                    want = (nfill * (p + 1)) // (npair + LAGP)
                    while fi < want:
                        fillers[fi]()
                        fi += 1
                while fi < nfill:
                    fillers[fi]()
                    fi += 1

            # ---- softmax normalize: per-partition 1/Z on the [q, hd]
            # blocks (Z is the PV-aug 129th column) ----
            def emit_c_norm(h, oz):
                atqs = []
                for b in range(4):
                    rz = rzp.tile([128, 1], FP32, name=f"rz{h}_{b}")
                    nc.vector.reciprocal_approx_fast(
                        rz[:], oz[:, b * 256 + HD:b * 256 + HD + 1])
                    atq = attp.tile([128, 128], BF16, name=f"atq{h}_{b}")
                    nc.vector.tensor_scalar_mul(
                        atq[:], oz[:, b * 256:b * 256 + HD], rz[:, 0:1])
                    atqs.append(atq)
                return atqs

            # ---- transpose the normalized [q, hd] blocks to [hd, q] for
            # the output projection (PE transposes via identity) ----
            def emit_at_T(h, atqs):
                tp = big.tile([128, 1024], FP32, name="atT", tag="big")
                ats = []
                for b in range(4):
                    nc.tensor.matmul(tp[:, b * 256:b * 256 + 128],
                                     atqs[b][:], ident_sb[:],
                                     start=True, stop=True)
                    at = attp.tile([128, 128], BF16, name=f"at{h}_{b}")
                    nc.vector.tensor_copy(at[:], tp[:, b * 256:b * 256 + 128])
                    ats.append(at)
                return ats

            # ---- output projection for chunk c (op tiles in the big pool,
            # drains on DVE, DMA alternating sync/gpsimd; K(c+1) projection
            # matmuls injected between op groups) ----
            def emit_d(c, ats0, ats1, fillers, split_dma=False):
                ats = [ats0, ats1]
                nfill = len(fillers)
                fi = 0
                for st4 in range(4):
                    st = 4 * c + st4
                    ost = ostp.tile([128, D], BF16, name="ost")
                    for dq in range(2):
                        op = big.tile([128, 1024], FP32, name="op", tag="big")
                        for hh in range(2):
                            for k2 in range(2):
                                dc = dq * 2 + k2
                                dsl = slice(dc * 512, (dc + 1) * 512)
                                nc.tensor.matmul(
                                    op[:, k2 * 512:(k2 + 1) * 512],
                                    ats[hh][st4][:],
                                    wot[hh][:, dsl], start=(hh == 0),
                                    stop=(hh == 1))
                        # steady-state drains all on DVE so the scalar
                        # queue holds only exps (chunk-start exps were
                        # stalling behind drains); last chunk splits both
                        # engines in parallel to shorten the tail
                        if split_dma and dq == 0:
                            nc.scalar.activation(
                                ost[:, dq * 1024:(dq + 1) * 1024], op[:],
                                AF.Copy)
                        else:
                            nc.vector.tensor_copy(
                                ost[:, dq * 1024:(dq + 1) * 1024], op[:])
                        if split_dma:
                            eng = nc.sync if (st4 + dq) % 2 == 0 else nc.scalar
                            eng.dma_start(
                                out=out[st * 128:(st + 1) * 128,
                                        dq * 1024:(dq + 1) * 1024],
                                in_=ost[:, dq * 1024:(dq + 1) * 1024])
                        # inject K-proj matmuls after the first op group
                        if st4 >= 1 or dq == 1:
                            want = (nfill * (2 * st4 + dq)) // 7
                            while fi < want:
                                fillers[fi]()
                                fi += 1
                    if not split_dma:
                        eng = nc.sync if st4 % 2 == 0 else nc.gpsimd
                        eng.dma_start(out=out[st * 128:(st + 1) * 128, :],
                                      in_=ost[:])
                while fi < nfill:
                    fillers[fi]()
                    fi += 1

            # ================= preamble: chunk 0 QKV + stats ==============
            istd4 = emit_stats_gram(0, ht8_cur)

            pp_q = ppp.tile([128, 1024], FP32, name="pp", tag="ppp")
            for f in qk_proj_fillers(pp_q, wq_sb, ht_cur):
                f()
            pp_k = ozA.tile([128, 1024], FP32, name="oz", tag="ozA")
            for f in qk_proj_fillers(pp_k, wk_sb, ht_cur):
                f()
            stats = emit_stats_bcast(0, istd4)
            istd4_n = emit_stats_gram(1, ht8_nxt)
            qt_c = emit_rope(0, pp_q, stats, "q")
            stats_n = emit_stats_bcast(1, istd4_n)
            pp_v = ppp.tile([128, 1024], FP32, name="pp", tag="ppp")
            emit_rope(0, pp_k, stats, "k")
            for f in v_proj_fillers(pp_v, ht_cur):
                f()
            emit_v_drain(0, pp_v, istd4)

            # ================= main loop ==================================
            # istd4_n/stats_n: chunk c+1; istd4_f: chunk c+2 (gram emitted
            # between the heads, bcast injected into head1's pair stream).
            istd4_f = stats_f = None
            for c in range(NCHUNK):
                last = (c == NCHUNK - 1)
                if c + 2 < NCHUNK:
                    ht8_fut = emit_ht8_dma(c + 2)
                    ht_fut = emit_ht_dma(c + 2)
                else:
                    ht8_fut = ht_fut = None

                # --- head 0 attention, with V(c+1) injected ---
                oz0 = ozA.tile([128, 1024], FP32, name="oz", tag="ozA")
                if not last:
                    pp_v = ppp.tile([128, 1024], FP32, name="pp", tag="ppp")
                    vfill = v_proj_fillers(pp_v, ht_nxt)
                else:
                    vfill = []
                emit_c_core(c, 0, qt_c, oz0, vfill)
                if not last:
                    emit_v_drain(c + 1, pp_v, istd4_n)
                atqs0 = emit_c_norm(0, oz0)
                if ht8_fut is not None:
                    istd4_f = emit_stats_gram(c + 2, ht8_fut)

                # --- head 1 attention, with at0-transpose + Q(c+1) +
                # bcast(c+2) injected into the pair stream ---
                oz1 = ozA.tile([128, 1024], FP32, name="oz", tag="ozA")
                if not last:
                    pp_q = ppp.tile([128, 1024], FP32, name="pp", tag="ppp")
                    qfill = qk_proj_fillers(pp_q, wq_sb, ht_nxt)
                else:
                    qfill = []
                a0holder = []

                def at0_fill(atqs0=atqs0, a0holder=a0holder):
                    a0holder.append(emit_at_T(0, atqs0))
                qfill = qfill[:2] + [at0_fill] + qfill[2:]
                if ht8_fut is not None:
                    holder = []

                    def bc_fill(istd4_f=istd4_f, c2=c + 2, holder=holder):
                        holder.append(emit_stats_bcast(c2, istd4_f))
                    qfill = qfill[:21] + [bc_fill] + qfill[21:]
                emit_c_core(c, 1, qt_c, oz1, qfill)
                ats0 = a0holder[0]
                atqs1 = emit_c_norm(1, oz1)
                ats1 = emit_at_T(1, atqs1)
                qt_next = None
                if not last:
                    qt_next = emit_rope(c + 1, pp_q, stats_n, "q")
                    pp_k = ozA.tile([128, 1024], FP32, name="oz", tag="ozA")
                    kfill = qk_proj_fillers(pp_k, wk_sb, ht_nxt)
                else:
                    kfill = []

                # --- output projection with K(c+1) injected ---
                emit_d(c, ats0, ats1, kfill, split_dma=last)
                if not last:
                    emit_rope(c + 1, pp_k, stats_n, "k")

                qt_c = qt_next
                istd4_n = istd4_f
                stats_n = holder[0] if ht8_fut is not None else None
                ht_nxt, ht8_nxt = ht_fut, ht8_fut

    nc.finalize()
    return nc


def _host_prep(xs, norm_w, wq, wk, wv, wo):
    """Fold norm_w into qkv weights, permute rope dims, build tables."""
    bf16 = ml_dtypes.bfloat16
    e4 = ml_dtypes.float8_e4m3
    nw = norm_w.astype(np.float32)[:, None, None]
    perm = np.concatenate([np.arange(0, HD, 2), np.arange(1, HD, 2)])
    wq_p = (wq * nw)[:, :, perm]
    wk_p = (wk * nw)[:, :, perm]
    wv_n = wv * nw

    inv_freq = 1.0 / (ROPE_BASE ** (np.arange(0, HD, 2, dtype=np.float32) / HD))
    pos = np.arange(SEQ, dtype=np.float32)
    ang = pos[:, None] * inv_freq[None, :]          # [S, 64]
    cos_t = np.cos(ang).T.astype(np.float32)        # [64, S]
    sin_t = np.sin(ang).T.astype(np.float32)
    cosd = np.concatenate([cos_t, cos_t], 0)        # [128, S]
    # [-sin; sin]: dst = pp*cos_sc + psw, psw[0:64] = pp[64:]*(-sin*istd),
    # psw[64:] = pp[0:64]*(sin*istd)
    sind = np.concatenate([-sin_t, sin_t], 0)

    trim = np.triu(np.ones((128, 128), dtype=np.float32))  # t <= s valid
    onesm = np.ones((128, 128), dtype=np.float32)
    identm = np.eye(128, dtype=np.float32)
    sel = np.kron(np.eye(4, dtype=np.float32), np.ones((1, 128), np.float32))

    xsT_f = np.ascontiguousarray(xs.astype(np.float32).T)  # [D, S]
    # chunk-major: [c, p, t, s] so each chunk tile is one contiguous DMA
    xcm = xsT_f.reshape(DT, 128, NCHUNK, CHUNK).transpose(2, 1, 0, 3)
    xcm = np.ascontiguousarray(xcm).reshape(NCHUNK * 128, DT * CHUNK)

    def wlay(w):  # [D, M] -> [128, DT*M] with d-tile-major columns
        return np.ascontiguousarray(
            w.reshape(DT, 128, M).transpose(1, 0, 2).reshape(128, DT * M)
            .astype(bf16))

    common = {
        "xsT": xcm.astype(bf16),
        "xsT8": np.clip(xcm * X8_SCALE, -240, 240).astype(e4),
        "cosd": np.ascontiguousarray(cosd.astype(bf16)),
        "sind": np.ascontiguousarray(sind.astype(bf16)),
        "tri": np.ascontiguousarray(trim.astype(bf16)),
        "ident": identm.astype(bf16),
        "identr": identm,
        "sel4": np.ascontiguousarray(sel),
    }
    in_maps = []
    for core in range(NCORES):
        h0 = core * HPC
        sl = slice(h0, h0 + HPC)
        in_maps.append({
            **common,
            "wq": wlay(wq_p[:, sl, :].reshape(D, M)),
            "wk": wlay(wk_p[:, sl, :].reshape(D, M)),
            "wv": wlay(wv_n[:, sl, :].reshape(D, M)),
            "wod": np.ascontiguousarray(
                wo[:, sl, :].reshape(D, M).T.astype(bf16)),
        })
    return in_maps


def kernel(xs, norm_w, wq, wk, wv, wo):
    trace = bool(int(os.environ.get("KERNEL_TRACE", "0")))
    if trace:
        _inject_ntff_hook()
    from concourse.bass_utils import run_bass_kernel_spmd

    nc = _build_nc()
    in_maps = _host_prep(np.asarray(xs), np.asarray(norm_w), np.asarray(wq),
                         np.asarray(wk), np.asarray(wv), np.asarray(wo))
    res = run_bass_kernel_spmd(nc, in_maps, core_ids=list(range(NCORES)),
                               trace=trace)
    if trace and res.exec_time_ns is not None:
        print(f"HW exec time: {res.exec_time_ns} ns")
    acc = np.zeros((SEQ, D), dtype=np.float32)
    for r in res.results:
        acc += r["out"].astype(np.float32)
    return acc


if __name__ == "__main__":
    rng = np.random.default_rng(0)
    scale = 1.0 / np.sqrt(D)
    inputs = {
        "xs": rng.standard_normal((SEQ, D), dtype=np.float32),
        "norm_w": np.ones((D,), np.float32),
        "wq": rng.standard_normal((D, NH, HD), dtype=np.float32) * scale,
        "wk": rng.standard_normal((D, NH, HD), dtype=np.float32) * scale,
        "wv": rng.standard_normal((D, NH, HD), dtype=np.float32) * scale,
        "wo": rng.standard_normal((D, NH, HD), dtype=np.float32) * scale,
    }
    out = kernel(**inputs)
    print(out.shape, out.dtype, float(np.abs(out).max()))
